# revision 18
# baseline (speedup 1.0000x reference)
"""Multi-head self-attention (B=2, N=4096, C=512, H=8) on 8 trn2 NeuronCores.

Sharding: one head per core (tensor parallel over heads). Each core:
  - computes Q^T,K^T (d-on-partitions, two batches packed on partition halves)
    via one merged [wq|wk] stationary (full 128-wide PE) + a DMA redistribute,
    and V (m-on-partitions) for its head from the full token stream,
  - runs flash-style attention per batch: S^T = K Q^T via row-tiled matmul
    pairs; the softmax exp is SPLIT between the scalar (ACT) engine (exact
    exp) and the vector engine (Schraudolph bitcast exp2: one tensor_scalar
    mult+add whose int16 output bits form the bf16 of 2^(A*s+B)), so both
    engines stream the N^2 softmax concurrently; AV matmuls run two key-chunks
    behind the exp so the PE never stalls on it,
  - P^T @ [V|1] accumulated in PSUM, the appended ones-column producing the
    softmax denominators for free,
  - normalizes the head output *before* projection (vector stt multiply with
    a gpsimd-broadcast reciprocal), projects through the head's w_proj slice
    with row-packed matmul pairs, and DMA-truncates the fp32 PSUM result
    straight to bf16 DRAM partials (truncation bias compensated in the
    normalize multiply).
Host sums the 8 bf16 partials in fp32 and adds b_proj.
"""

import numpy as np
import ml_dtypes

import concourse.bacc as bacc
import concourse.bass as bass
import concourse.mybir as mybir
import concourse.tile as tile
from concourse.bass_utils import run_bass_kernel_spmd

BF16 = ml_dtypes.bfloat16

B = 2
N = 4096          # sequence length per batch
C = 512           # channels
H = 8             # heads
DH = C // H       # 64 head dim
T = B * N         # total tokens
NB = 512          # query-block size
MC = 128          # key-chunk size
SCALE = float(DH) ** -0.5

# Schraudolph bitcast exp2 constants (bf16 target):
#   bf16 bits of exp(SCALE*s)  ~=  round(SCH_A*s + SCH_B)
SCH_A = 128.0 * SCALE * 1.4426950408889634
SCH_B = 16256.0 - 128.0 * 0.045

# engine casts round; no truncation compensation needed
TRUNC_COMP = 1.0

N_ACT = 18        # of every 32 key chunks, this many exp'd on ACT (rest DVE)


def _act_mask(n_act, n=32):
    return [((mc + 1) * n_act) // n - (mc * n_act) // n > 0 for mc in range(n)]


def _attention_body(nc, tc, xt, wqk, wv, wp2, out, n_seq):
    dt = mybir.dt
    cch = C // 128             # contraction chunks over C
    nblk = n_seq // NB         # query blocks per batch
    nmc = n_seq // MC          # key chunks per batch
    ntc = n_seq // 512         # 512-token chunks per batch (qkv prep)
    tpb = NB // 128            # 128-token proj chunks per query block
    EXP = mybir.ActivationFunctionType.Exp
    MUL = mybir.AluOpType.mult
    ADD = mybir.AluOpType.add
    act_mask = _act_mask(N_ACT if nmc == 32 else max(1, int(round(N_ACT * nmc / 32.0))),
                         nmc)

    const = tc.alloc_tile_pool(name="const", bufs=1)
    persist = tc.alloc_tile_pool(name="persist", bufs=1)

    # constants
    wqk_sb = const.tile([128, cch, 128], dt.bfloat16)
    wv_sb = const.tile([128, cch, DH], dt.bfloat16)
    nc.sync.dma_start(wqk_sb[:], wqk.rearrange("(c p) d -> p c d", p=128))
    nc.sync.dma_start(wv_sb[:], wv.rearrange("(c p) d -> p c d", p=128))
    wp_sb = const.tile([128, C], dt.bfloat16)     # wp duplicated on both halves
    nc.sync.dma_start(wp_sb[:], wp2)
    ones_sb = const.tile([128, 1], dt.bfloat16)   # denominator matmul column
    nc.vector.memset(ones_sb[:], 1.0)

    # persistent per-head tensors
    qt2 = persist.tile([128, n_seq], dt.bfloat16)   # rows 0:64 b0 Q^T, 64:128 b1
    kt2 = persist.tile([128, n_seq], dt.bfloat16)
    vext = [persist.tile([128, nmc * DH], dt.bfloat16, name=f"vext{j}")
            for j in range(2)]
    otsb = persist.tile([128, n_seq], dt.bfloat16)  # rows 0:64 b0 head-out, 64:128 b1

    # fused structure: QKV prep is interleaved into query-block 0's attention
    # stream so the exp engines start working a few µs in; the prep PSUM pools
    # close before the first projection so their space is recycled for it.
    with tc.tile_pool(name="s_ps", bufs=2, space="PSUM") as sps, \
         tc.tile_pool(name="acc_ps", bufs=1, space="PSUM") as aps, \
         tc.tile_pool(name="ptp", bufs=4) as ptp, \
         tc.tile_pool(name="obp", bufs=2) as obp, \
         tc.tile_pool(name="rrp", bufs=2) as rrp:

        def emit_prep(c, xpool, stgp, pps, vps):
            """QKV prep for one 512-token chunk of both batches."""
            xab = []
            for half in range(2):
                xa = xpool.tile([128, cch, 512], dt.bfloat16, tag="x")
                nc.sync.dma_start(
                    xa[:], xt[:, half * n_seq + c * 512:half * n_seq + (c + 1) * 512]
                    .rearrange("(k p) i -> p k i", p=128))
                xab.append(xa)
            for half, xa in enumerate(xab):
                # merged [Q^T | K^T] for this batch: full 128-wide stationary
                ps = pps.tile([128, 512], dt.float32, tag="qk")
                for k in range(cch):
                    nc.tensor.matmul(ps[:], wqk_sb[:, k, :], xa[:, k, :],
                                     start=(k == 0), stop=(k == cch - 1))
                stg = stgp.tile([128, 512], dt.bfloat16, tag="stg")
                nc.vector.tensor_copy(stg[:], ps[:])
                # redistribute to batch-packed partition halves (cross-partition)
                nc.sync.dma_start(qt2[half * DH:(half + 1) * DH,
                                      c * 512:(c + 1) * 512], stg[0:DH, :])
                nc.sync.dma_start(kt2[half * DH:(half + 1) * DH,
                                      c * 512:(c + 1) * 512], stg[DH:128, :])
                # V: [m, d] tiles, one per 128 tokens
                psv = vps.tile([128, 4, DH], dt.float32, tag="v")
                for mt in range(4):
                    for k in range(cch):
                        nc.tensor.matmul(psv[:, mt, :],
                                         xa[:, k, mt * 128:(mt + 1) * 128],
                                         wv_sb[:, k, :],
                                         start=(k == 0), stop=(k == cch - 1))
                nc.vector.tensor_copy(
                    vext[half][:, c * 4 * DH:(c + 1) * 4 * DH],
                    psv[:, :, :])

        def emit_proj(nb, jps):
            """Projection + bf16 store for query block nb."""
            for t in range(tpb):
                gt = nb * tpb + t
                pp = [jps.tile([128, C], dt.float32, tag=f"pp{j}", name=f"pp{j}")
                      for j in range(2)]
                nc.tensor.matmul(pp[0][:], otsb[0:DH, gt * 128:(gt + 1) * 128],
                                 wp_sb[0:DH, :], start=True, stop=True,
                                 tile_position=(0, 0))
                nc.tensor.matmul(pp[1][:], otsb[DH:128, gt * 128:(gt + 1) * 128],
                                 wp_sb[DH:128, :], start=True, stop=True,
                                 tile_position=(64, 0))
                for j in range(2):
                    ob = obp.tile([128, C], dt.bfloat16, tag=f"ob{j}", name="ob")
                    if j == 0:
                        nc.vector.tensor_copy(ob[:], pp[j][:])
                    else:
                        nc.scalar.copy(ob[:], pp[j][:])
                    nc.sync.dma_start(
                        out[j * n_seq + gt * 128: j * n_seq + (gt + 1) * 128, :],
                        ob[:])

        def emit_norm(nb, acc, dbs):
            """Deferred DVE half of block nb's epilogue: 1/D + normalize into
            otsb. Emitted a couple of key-chunks into block nb+1 so the DVE
            FIFO never stalls on the gpsimd broadcast round trip."""
            for j in range(2):
                rr = rrp.tile([DH, NB], dt.float32, tag="rr", name="rr")
                nc.vector.reciprocal_approx_fast(rr[:], dbs[j][:])
                nc.vector.scalar_tensor_tensor(
                    otsb[j * DH:(j + 1) * DH, nb * NB:(nb + 1) * NB],
                    acc[j * DH:(j + 1) * DH, :], TRUNC_COMP, rr[:], MUL, MUL)

        def emit_block(nb, prev, jps, prep_sched=None):
            """One query block's S^T/exp/AV stream with deferred epilogues.
            prep_sched: optional {mc: chunk} map of prep work to interleave."""
            acc = aps.tile([128, NB], dt.float32, tag="acc", name="acc")
            accD = aps.tile([DH + 1, NB], dt.float32, tag="accD", name="accD")
            # AV runs TWO steps behind S^T/exp so the PE never waits on exp.
            pending = []   # [(pt_tile, mc), ...]
            for mc in range(nmc):
                st = sps.tile([128, 1024], dt.float32, tag="s")
                for j in range(2):
                    nc.tensor.matmul(
                        st[:, j * 512:j * 512 + NB],
                        kt2[j * DH:(j + 1) * DH, mc * 128:(mc + 1) * 128],
                        qt2[j * DH:(j + 1) * DH, nb * NB:(nb + 1) * NB],
                        start=True, stop=True,
                        tile_position=(j * 64, 0))
                pt = ptp.tile([128, 1024], dt.bfloat16, tag="pt")
                if act_mask[mc]:
                    nc.scalar.activation(pt[:], st[:], EXP, bias=0.0, scale=SCALE)
                else:
                    nc.vector.tensor_scalar(pt[:].bitcast(dt.int16), st[:],
                                            SCH_A, SCH_B, MUL, ADD)
                pending.append((pt, mc))
                if len(pending) > 2 or (mc == nmc - 1):
                    todo = pending if mc == nmc - 1 else [pending.pop(0)]
                    for ppt, pmc in todo:
                        st0, sp1 = (pmc == 0), (pmc == nmc - 1)
                        vsl = slice(pmc * DH, (pmc + 1) * DH)
                        # slot A: AV j0 (cols 0-63) ∥ D j0 (ones at col 64)
                        nc.tensor.matmul(acc[0:DH, :], vext[0][:, vsl],
                                         ppt[:, 0:NB], start=st0, stop=sp1,
                                         tile_position=(0, 0))
                        nc.tensor.matmul(accD[DH:DH + 1, :], ones_sb[:],
                                         ppt[:, 0:NB], start=st0, stop=sp1,
                                         tile_position=(0, 64))
                        # slot B: D j1 (ones at col 0) ∥ AV j1 (cols 64-127)
                        nc.tensor.matmul(accD[0:1, :], ones_sb[:],
                                         ppt[:, 512:512 + NB], start=st0,
                                         stop=sp1, tile_position=(0, 0))
                        nc.tensor.matmul(acc[DH:128, :], vext[1][:, vsl],
                                         ppt[:, 512:512 + NB], start=st0,
                                         stop=sp1, tile_position=(0, 64))
                if prep_sched and mc in prep_sched:
                    emit_prep(*prep_sched[mc])
                if mc == 1 and prev is not None:
                    emit_norm(*prev)    # previous block's normalize, deferred
                    prev = None
                if mc == 5 and nb > 0:
                    emit_proj(nb - 1, jps)  # previous block's projection
            # epilogue (ACT + gpsimd legs now; DVE legs deferred into nb+1).
            # j0's denominator row sits at partition 64 (col-tile position) —
            # a tiny DMA rehomes it to partition 0 for the broadcast.
            dbs = []
            for j in range(2):
                db = rrp.tile([DH, NB], dt.float32, tag="db", name="db")
                if j == 0:
                    dsA = rrp.tile([DH + 1, NB], dt.float32, tag="dsA", name="dsA")
                    dsb = rrp.tile([1, NB], dt.float32, tag="dsb0", name="dsb0")
                    nc.scalar.copy(dsA[DH:DH + 1, :], accD[DH:DH + 1, :])
                    nc.sync.dma_start(dsb[:], dsA[DH:DH + 1, :])
                else:
                    dsb = rrp.tile([1, NB], dt.float32, tag="dsb1", name="dsb1")
                    nc.scalar.copy(dsb[:], accD[0:1, :])
                nc.gpsimd.partition_broadcast(db[:], dsb[:])
                dbs.append(db)
            return (nb, acc, dbs)

        with tc.tile_pool(name="xa", bufs=4) as xpool, \
             tc.tile_pool(name="stg", bufs=3) as stgp, \
             tc.tile_pool(name="prep_ps", bufs=1, space="PSUM") as pps, \
             tc.tile_pool(name="prep_v_ps", bufs=1, space="PSUM") as vps:
            pools = (xpool, stgp, pps, vps)
            for c in range(2):
                emit_prep(c, *pools)
            sched = {3 + 4 * i: (2 + i,) + pools for i in range(ntc - 2)}
            prev = emit_block(0, None, None, prep_sched=sched)
        with tc.tile_pool(name="proj_ps", bufs=1, space="PSUM") as jps:
            for nb in range(1, nblk):
                prev = emit_block(nb, prev, jps)
            emit_norm(*prev)
            emit_proj(nblk - 1, jps)

    persist.release()
    const.release()


def build_kernel(n_seq=N):
    nc = bacc.Bacc("TRN2", target_bir_lowering=False, debug=False, num_devices=8)
    dt = mybir.dt
    t_tot = 2 * n_seq
    xt = nc.dram_tensor("xt", [C, t_tot], dt.bfloat16, kind="ExternalInput").ap()
    wqk = nc.dram_tensor("wqk", [C, 128], dt.bfloat16, kind="ExternalInput").ap()
    wv = nc.dram_tensor("wv", [C, DH], dt.bfloat16, kind="ExternalInput").ap()
    wp2 = nc.dram_tensor("wp2", [128, C], dt.bfloat16, kind="ExternalInput").ap()
    out = nc.dram_tensor("out", [t_tot, C], dt.bfloat16, kind="ExternalOutput").ap()
    with tile.TileContext(nc) as tc:
        _attention_body(nc, tc, xt, wqk, wv, wp2, out, n_seq)
    nc.compile()
    return nc


def make_in_maps(x, w_qkv, w_proj, n_seq=N):
    """Slice the full inputs into 8 per-core input maps (head per core)."""
    t_tot = 2 * n_seq
    xt = np.ascontiguousarray(x.reshape(t_tot, C).T).astype(BF16)
    in_maps = []
    for h in range(H):
        wq = w_qkv[h * DH:(h + 1) * DH, :].T                      # [C, DH]
        wk = w_qkv[C + h * DH:C + (h + 1) * DH, :].T
        wqk = np.ascontiguousarray(
            np.concatenate([wq, wk], axis=1)).astype(BF16)        # [C, 128]
        wv = np.ascontiguousarray(
            w_qkv[2 * C + h * DH:2 * C + (h + 1) * DH, :].T).astype(BF16)
        wp = np.ascontiguousarray(w_proj[:, h * DH:(h + 1) * DH].T)  # [DH, C]
        wp2 = np.concatenate([wp, wp], axis=0).astype(BF16)          # [128, C]
        in_maps.append({"xt": xt, "wqk": wqk, "wv": wv, "wp2": wp2})
    return in_maps


_NC_CACHE = {}


def _get_nc(n_seq=N):
    if n_seq not in _NC_CACHE:
        _NC_CACHE[n_seq] = build_kernel(n_seq)
    return _NC_CACHE[n_seq]


def run(x, w_qkv, w_proj, b_proj, trace=False, tmpdir=None):
    x = np.asarray(x, dtype=np.float32)
    w_qkv = np.asarray(w_qkv, dtype=np.float32)
    w_proj = np.asarray(w_proj, dtype=np.float32)
    b_proj = np.asarray(b_proj, dtype=np.float32)
    nc = _get_nc()
    in_maps = make_in_maps(x, w_qkv, w_proj)
    try:
        res = run_bass_kernel_spmd(nc, in_maps, list(range(H)), trace=trace,
                                   tmpdir=tmpdir)
    except ModuleNotFoundError:
        res = run_bass_kernel_spmd(nc, in_maps, list(range(H)), trace=False,
                                   tmpdir=tmpdir)
    partial_sum = np.zeros((T, C), np.float32)
    for r in res.results:
        partial_sum += r["out"].astype(np.float32)
    full = partial_sum + b_proj[None, :]
    return full.reshape(B, N, C), res


def kernel(x, w_qkv, w_proj, b_proj):
    out, _ = run(x, w_qkv, w_proj, b_proj)
    return out


# revision 19
# speedup vs baseline: 1.4922x; 1.4922x over previous
"""Multi-head self-attention (B=2, N=4096, C=512, H=8) on 8 trn2 NeuronCores.

Sharding: one head per core (tensor parallel over heads). Each core:
  - computes Q^T,K^T (d-on-partitions, two batches packed on partition halves)
    via one merged [wq|wk] stationary (full 128-wide PE) + a DMA redistribute,
    and V (m-on-partitions) for its head from the full token stream,
  - runs flash-style attention per batch: S^T = K Q^T via row-tiled matmul
    pairs; the softmax exp is SPLIT between the scalar (ACT) engine (exact
    exp) and the vector engine (Schraudolph bitcast exp2: one tensor_scalar
    mult+add whose int16 output bits form the bf16 of 2^(A*s+B)), so both
    engines stream the N^2 softmax concurrently; AV matmuls run two key-chunks
    behind the exp so the PE never stalls on it,
  - P^T @ [V|1] accumulated in PSUM, the appended ones-column producing the
    softmax denominators for free,
  - normalizes the head output *before* projection (vector stt multiply with
    a gpsimd-broadcast reciprocal), projects through the head's w_proj slice
    with row-packed matmul pairs, and DMA-truncates the fp32 PSUM result
    straight to bf16 DRAM partials (truncation bias compensated in the
    normalize multiply).
Host sums the 8 bf16 partials in fp32 and adds b_proj.
"""

import numpy as np
import ml_dtypes

import concourse.bacc as bacc
import concourse.bass as bass
import concourse.mybir as mybir
import concourse.tile as tile
from concourse.bass_utils import run_bass_kernel_spmd

BF16 = ml_dtypes.bfloat16

B = 2
N = 4096          # sequence length per batch
C = 512           # channels
H = 8             # heads
DH = C // H       # 64 head dim
T = B * N         # total tokens
NB = 512          # query-block size
MC = 128          # key-chunk size
SCALE = float(DH) ** -0.5

# Schraudolph bitcast exp2 constants (bf16 target):
#   bf16 bits of exp(SCALE*s)  ~=  round(SCH_A*s + SCH_B)
SCH_A = 128.0 * SCALE * 1.4426950408889634
SCH_B = 16256.0 - 128.0 * 0.045

# engine casts round; no truncation compensation needed
TRUNC_COMP = 1.0

N_ACT = 18        # of every 32 key chunks, this many exp'd on ACT (rest DVE)


def _act_mask(n_act, n=32):
    return [((mc + 1) * n_act) // n - (mc * n_act) // n > 0 for mc in range(n)]


def _attention_body(nc, tc, xt, wqk, wv, wp2, out, n_seq):
    dt = mybir.dt
    cch = C // 128             # contraction chunks over C
    nblk = n_seq // NB         # query blocks per batch
    nmc = n_seq // MC          # key chunks per batch
    ntc = n_seq // 512         # 512-token chunks per batch (qkv prep)
    tpb = NB // 128            # 128-token proj chunks per query block
    EXP = mybir.ActivationFunctionType.Exp
    MUL = mybir.AluOpType.mult
    ADD = mybir.AluOpType.add
    act_mask = _act_mask(N_ACT if nmc == 32 else max(1, int(round(N_ACT * nmc / 32.0))),
                         nmc)

    const = tc.alloc_tile_pool(name="const", bufs=1)
    persist = tc.alloc_tile_pool(name="persist", bufs=1)

    # constants
    wqk_sb = const.tile([128, cch, 128], dt.bfloat16)
    wv_sb = const.tile([128, cch, DH], dt.bfloat16)
    nc.sync.dma_start(wqk_sb[:], wqk.rearrange("(c p) d -> p c d", p=128))
    nc.sync.dma_start(wv_sb[:], wv.rearrange("(c p) d -> p c d", p=128))
    wp_sb = const.tile([128, C], dt.bfloat16)     # wp duplicated on both halves
    nc.sync.dma_start(wp_sb[:], wp2)

    # persistent per-head tensors
    qt2 = persist.tile([128, n_seq], dt.bfloat16)   # rows 0:64 b0 Q^T, 64:128 b1
    kt2 = persist.tile([128, n_seq], dt.bfloat16)
    vext = [persist.tile([128, nmc * (DH + 1)], dt.bfloat16, name=f"vext{j}")
            for j in range(2)]
    otsb = persist.tile([128, n_seq], dt.bfloat16)  # rows 0:64 b0 head-out, 64:128 b1

    # fused structure: QKV prep is interleaved into query-block 0's attention
    # stream so the exp engines start working a few µs in; the prep PSUM pools
    # close before the first projection so their space is recycled for it.
    with tc.tile_pool(name="s_ps", bufs=2, space="PSUM") as sps, \
         tc.tile_pool(name="acc_ps", bufs=1, space="PSUM") as aps, \
         tc.tile_pool(name="ptp", bufs=4) as ptp, \
         tc.tile_pool(name="obp", bufs=2) as obp, \
         tc.tile_pool(name="rrp", bufs=2) as rrp:

        def emit_prep(c, xpool, stgp, pps, vps):
            """QKV prep for one 512-token chunk of both batches."""
            xab = []
            for half in range(2):
                xa = xpool.tile([128, cch, 512], dt.bfloat16, tag="x")
                nc.sync.dma_start(
                    xa[:], xt[:, half * n_seq + c * 512:half * n_seq + (c + 1) * 512]
                    .rearrange("(k p) i -> p k i", p=128))
                xab.append(xa)
            for half, xa in enumerate(xab):
                # merged [Q^T | K^T] for this batch: full 128-wide stationary
                ps = pps.tile([128, 512], dt.float32, tag="qk")
                for k in range(cch):
                    nc.tensor.matmul(ps[:], wqk_sb[:, k, :], xa[:, k, :],
                                     start=(k == 0), stop=(k == cch - 1))
                stg = stgp.tile([128, 512], dt.bfloat16, tag="stg")
                nc.vector.tensor_copy(stg[:], ps[:])
                # redistribute to batch-packed partition halves (cross-partition)
                nc.sync.dma_start(qt2[half * DH:(half + 1) * DH,
                                      c * 512:(c + 1) * 512], stg[0:DH, :])
                nc.sync.dma_start(kt2[half * DH:(half + 1) * DH,
                                      c * 512:(c + 1) * 512], stg[DH:128, :])
                # V: [m, d] tiles, one per 128 tokens; ones column appended
                psv = vps.tile([128, 4, DH + 2], dt.float32, tag="v")
                for mt in range(4):
                    for k in range(cch):
                        nc.tensor.matmul(psv[:, mt, 0:DH],
                                         xa[:, k, mt * 128:(mt + 1) * 128],
                                         wv_sb[:, k, :],
                                         start=(k == 0), stop=(k == cch - 1))
                nc.vector.memset(psv[:, :, DH:DH + 1], 1.0)
                nc.vector.tensor_copy(
                    vext[half][:].rearrange(
                        "p (t c) -> p t c", c=DH + 1)[:, c * 4:(c + 1) * 4, :],
                    psv[:, :, 0:DH + 1])

        def emit_proj(nb, jps):
            """Projection + bf16 store for query block nb."""
            for t in range(tpb):
                gt = nb * tpb + t
                pp = [jps.tile([128, C], dt.float32, tag=f"pp{j}", name=f"pp{j}")
                      for j in range(2)]
                nc.tensor.matmul(pp[0][:], otsb[0:DH, gt * 128:(gt + 1) * 128],
                                 wp_sb[0:DH, :], start=True, stop=True,
                                 tile_position=(0, 0))
                nc.tensor.matmul(pp[1][:], otsb[DH:128, gt * 128:(gt + 1) * 128],
                                 wp_sb[DH:128, :], start=True, stop=True,
                                 tile_position=(64, 0))
                for j in range(2):
                    ob = obp.tile([128, C], dt.bfloat16, tag=f"ob{j}", name="ob")
                    if j == 0:
                        nc.vector.tensor_copy(ob[:], pp[j][:])
                    else:
                        nc.scalar.copy(ob[:], pp[j][:])
                    nc.sync.dma_start(
                        out[j * n_seq + gt * 128: j * n_seq + (gt + 1) * 128, :],
                        ob[:])

        def emit_norm(nb, accp, dbs):
            """Deferred DVE half of block nb's epilogue: 1/D + normalize into
            otsb. Emitted a couple of key-chunks into block nb+1 so the DVE
            FIFO never stalls on the gpsimd broadcast round trip."""
            for j in range(2):
                rr = rrp.tile([DH, NB], dt.float32, tag="rr", name="rr")
                nc.vector.reciprocal_approx_fast(rr[:], dbs[j][:])
                nc.vector.scalar_tensor_tensor(
                    otsb[j * DH:(j + 1) * DH, nb * NB:(nb + 1) * NB],
                    accp[j][0:DH, :], TRUNC_COMP, rr[:], MUL, MUL)

        def emit_block(nb, prev, jps, prep_sched=None):
            """One query block's S^T/exp/AV stream with deferred epilogues.
            prep_sched: optional {mc: chunk} map of prep work to interleave."""
            acc = [aps.tile([DH + 1, NB], dt.float32, tag=f"acc{j}",
                            name=f"acc{j}") for j in range(2)]
            # AV runs TWO steps behind S^T/exp so the PE never waits on exp.
            pending = []   # [(pt_tile, mc), ...]
            for mc in range(nmc):
                st = sps.tile([128, 1024], dt.float32, tag="s")
                for j in range(2):
                    nc.tensor.matmul(
                        st[:, j * 512:j * 512 + NB],
                        kt2[j * DH:(j + 1) * DH, mc * 128:(mc + 1) * 128],
                        qt2[j * DH:(j + 1) * DH, nb * NB:(nb + 1) * NB],
                        start=True, stop=True,
                        tile_position=(j * 64, 0))
                pt = ptp.tile([128, 1024], dt.bfloat16, tag="pt")
                if act_mask[mc]:
                    nc.scalar.activation(pt[:], st[:], EXP, bias=0.0, scale=SCALE)
                else:
                    nc.vector.tensor_scalar(pt[:].bitcast(dt.int16), st[:],
                                            SCH_A, SCH_B, MUL, ADD)
                pending.append((pt, mc))
                if len(pending) > 2 or (mc == nmc - 1):
                    todo = pending if mc == nmc - 1 else [pending.pop(0)]
                    for ppt, pmc in todo:
                        for j in range(2):
                            nc.tensor.matmul(
                                acc[j][:],
                                vext[j][:, pmc * (DH + 1):(pmc + 1) * (DH + 1)],
                                ppt[:, j * 512:j * 512 + NB],
                                start=(pmc == 0), stop=(pmc == nmc - 1))
                if prep_sched and mc in prep_sched:
                    emit_prep(*prep_sched[mc])
                if mc == 1 and prev is not None:
                    emit_norm(*prev)    # previous block's normalize, deferred
                    prev = None
                if mc == 5 and nb > 0:
                    emit_proj(nb - 1, jps)  # previous block's projection
            # epilogue (ACT + gpsimd legs now; DVE legs deferred into nb+1)
            dbs = []
            for j in range(2):
                dsb = rrp.tile([1, NB], dt.float32, tag="dsb", name="dsb")
                db = rrp.tile([DH, NB], dt.float32, tag="db", name="db")
                nc.scalar.copy(dsb[:], acc[j][DH:DH + 1, :])
                nc.gpsimd.partition_broadcast(db[:], dsb[:])
                dbs.append(db)
            return (nb, acc, dbs)

        with tc.tile_pool(name="xa", bufs=4) as xpool, \
             tc.tile_pool(name="stg", bufs=3) as stgp, \
             tc.tile_pool(name="prep_ps", bufs=1, space="PSUM") as pps, \
             tc.tile_pool(name="prep_v_ps", bufs=1, space="PSUM") as vps:
            pools = (xpool, stgp, pps, vps)
            for c in range(2):
                emit_prep(c, *pools)
            sched = {3 + 4 * i: (2 + i,) + pools for i in range(ntc - 2)}
            prev = emit_block(0, None, None, prep_sched=sched)
        with tc.tile_pool(name="proj_ps", bufs=1, space="PSUM") as jps:
            for nb in range(1, nblk):
                prev = emit_block(nb, prev, jps)
            emit_norm(*prev)
            emit_proj(nblk - 1, jps)

    persist.release()
    const.release()


def build_kernel(n_seq=N):
    nc = bacc.Bacc("TRN2", target_bir_lowering=False, debug=False, num_devices=8)
    dt = mybir.dt
    t_tot = 2 * n_seq
    xt = nc.dram_tensor("xt", [C, t_tot], dt.bfloat16, kind="ExternalInput").ap()
    wqk = nc.dram_tensor("wqk", [C, 128], dt.bfloat16, kind="ExternalInput").ap()
    wv = nc.dram_tensor("wv", [C, DH], dt.bfloat16, kind="ExternalInput").ap()
    wp2 = nc.dram_tensor("wp2", [128, C], dt.bfloat16, kind="ExternalInput").ap()
    out = nc.dram_tensor("out", [t_tot, C], dt.bfloat16, kind="ExternalOutput").ap()
    with tile.TileContext(nc) as tc:
        _attention_body(nc, tc, xt, wqk, wv, wp2, out, n_seq)
    nc.compile()
    return nc


def make_in_maps(x, w_qkv, w_proj, n_seq=N):
    """Slice the full inputs into 8 per-core input maps (head per core)."""
    t_tot = 2 * n_seq
    xt = np.ascontiguousarray(x.reshape(t_tot, C).T).astype(BF16)
    in_maps = []
    for h in range(H):
        wq = w_qkv[h * DH:(h + 1) * DH, :].T                      # [C, DH]
        wk = w_qkv[C + h * DH:C + (h + 1) * DH, :].T
        wqk = np.ascontiguousarray(
            np.concatenate([wq, wk], axis=1)).astype(BF16)        # [C, 128]
        wv = np.ascontiguousarray(
            w_qkv[2 * C + h * DH:2 * C + (h + 1) * DH, :].T).astype(BF16)
        wp = np.ascontiguousarray(w_proj[:, h * DH:(h + 1) * DH].T)  # [DH, C]
        wp2 = np.concatenate([wp, wp], axis=0).astype(BF16)          # [128, C]
        in_maps.append({"xt": xt, "wqk": wqk, "wv": wv, "wp2": wp2})
    return in_maps


_NC_CACHE = {}


def _get_nc(n_seq=N):
    if n_seq not in _NC_CACHE:
        _NC_CACHE[n_seq] = build_kernel(n_seq)
    return _NC_CACHE[n_seq]


def run(x, w_qkv, w_proj, b_proj, trace=False, tmpdir=None):
    x = np.asarray(x, dtype=np.float32)
    w_qkv = np.asarray(w_qkv, dtype=np.float32)
    w_proj = np.asarray(w_proj, dtype=np.float32)
    b_proj = np.asarray(b_proj, dtype=np.float32)
    nc = _get_nc()
    in_maps = make_in_maps(x, w_qkv, w_proj)
    try:
        res = run_bass_kernel_spmd(nc, in_maps, list(range(H)), trace=trace,
                                   tmpdir=tmpdir)
    except ModuleNotFoundError:
        res = run_bass_kernel_spmd(nc, in_maps, list(range(H)), trace=False,
                                   tmpdir=tmpdir)
    partial_sum = np.zeros((T, C), np.float32)
    for r in res.results:
        partial_sum += r["out"].astype(np.float32)
    full = partial_sum + b_proj[None, :]
    return full.reshape(B, N, C), res


def kernel(x, w_qkv, w_proj, b_proj):
    out, _ = run(x, w_qkv, w_proj, b_proj)
    return out


# revision 22
# speedup vs baseline: 1.4936x; 1.0009x over previous
"""Multi-head self-attention (B=2, N=4096, C=512, H=8) on 8 trn2 NeuronCores.

Sharding: one head per core (tensor parallel over heads). Each core:
  - computes Q^T,K^T (d-on-partitions, two batches packed on partition halves)
    via one merged [wq|wk] stationary (full 128-wide PE) + a DMA redistribute,
    and V (m-on-partitions) for its head from the full token stream,
  - runs flash-style attention per batch: S^T = K Q^T via row-tiled matmul
    pairs; the softmax exp is SPLIT between the scalar (ACT) engine (exact
    exp) and the vector engine (Schraudolph bitcast exp2: one tensor_scalar
    mult+add whose int16 output bits form the bf16 of 2^(A*s+B)), so both
    engines stream the N^2 softmax concurrently; AV matmuls run two key-chunks
    behind the exp so the PE never stalls on it,
  - P^T @ [V|1] accumulated in PSUM, the appended ones-column producing the
    softmax denominators for free,
  - normalizes the head output *before* projection (vector stt multiply with
    a gpsimd-broadcast reciprocal), projects through the head's w_proj slice
    with row-packed matmul pairs, and DMA-truncates the fp32 PSUM result
    straight to bf16 DRAM partials (truncation bias compensated in the
    normalize multiply).
Host sums the 8 bf16 partials in fp32 and adds b_proj.
"""

import numpy as np
import ml_dtypes

import concourse.bacc as bacc
import concourse.bass as bass
import concourse.mybir as mybir
import concourse.tile as tile
from concourse.bass_utils import run_bass_kernel_spmd

BF16 = ml_dtypes.bfloat16

B = 2
N = 4096          # sequence length per batch
C = 512           # channels
H = 8             # heads
DH = C // H       # 64 head dim
T = B * N         # total tokens
NB = 512          # query-block size
MC = 128          # key-chunk size
SCALE = float(DH) ** -0.5

# Schraudolph bitcast exp2 constants (bf16 target):
#   bf16 bits of exp(SCALE*s)  ~=  round(SCH_A*s + SCH_B)
SCH_A = 128.0 * SCALE * 1.4426950408889634
SCH_B = 16256.0 - 128.0 * 0.045

# engine casts round; no truncation compensation needed
TRUNC_COMP = 1.0

N_ACT = 18        # of every 32 key chunks, this many exp'd on ACT (rest DVE)


def _act_mask(n_act, n=32):
    return [((mc + 1) * n_act) // n - (mc * n_act) // n > 0 for mc in range(n)]


def _attention_body(nc, tc, xt, wqk, wv, wp2, out, n_seq):
    dt = mybir.dt
    cch = C // 128             # contraction chunks over C
    nblk = n_seq // NB         # query blocks per batch
    nmc = n_seq // MC          # key chunks per batch
    ntc = n_seq // 512         # 512-token chunks per batch (qkv prep)
    tpb = NB // 128            # 128-token proj chunks per query block
    EXP = mybir.ActivationFunctionType.Exp
    MUL = mybir.AluOpType.mult
    ADD = mybir.AluOpType.add
    act_mask = _act_mask(N_ACT if nmc == 32 else max(1, int(round(N_ACT * nmc / 32.0))),
                         nmc)

    const = tc.alloc_tile_pool(name="const", bufs=1)
    persist = tc.alloc_tile_pool(name="persist", bufs=1)

    # constants
    wqk_sb = const.tile([128, cch, 128], dt.bfloat16)
    wv_sb = const.tile([128, cch, DH], dt.bfloat16)
    nc.sync.dma_start(wqk_sb[:], wqk.rearrange("(c p) d -> p c d", p=128))
    nc.sync.dma_start(wv_sb[:], wv.rearrange("(c p) d -> p c d", p=128))
    wp_sb = const.tile([128, C], dt.bfloat16)     # wp duplicated on both halves
    nc.sync.dma_start(wp_sb[:], wp2)

    # persistent per-head tensors
    qt2 = persist.tile([128, n_seq], dt.bfloat16)   # rows 0:64 b0 Q^T, 64:128 b1
    kt2 = persist.tile([128, n_seq], dt.bfloat16)
    vext = [persist.tile([128, nmc * (DH + 1)], dt.bfloat16, name=f"vext{j}")
            for j in range(2)]
    otsb = persist.tile([128, n_seq], dt.bfloat16)  # rows 0:64 b0 head-out, 64:128 b1

    # fused structure: QKV prep is interleaved into query-block 0's attention
    # stream so the exp engines start working a few µs in; the prep PSUM pools
    # close before the first projection so their space is recycled for it.
    with tc.tile_pool(name="s_ps", bufs=2, space="PSUM") as sps, \
         tc.tile_pool(name="acc_ps", bufs=1, space="PSUM") as aps, \
         tc.tile_pool(name="ptp", bufs=7) as ptp, \
         tc.tile_pool(name="obp", bufs=2) as obp, \
         tc.tile_pool(name="rrp", bufs=2) as rrp:

        def emit_prep(c, xpool, stgp, pps, vps):
            """QKV prep for one 512-token chunk of both batches."""
            xab = []
            for half in range(2):
                xa = xpool.tile([128, cch, 512], dt.bfloat16, tag="x")
                nc.sync.dma_start(
                    xa[:], xt[:, half * n_seq + c * 512:half * n_seq + (c + 1) * 512]
                    .rearrange("(k p) i -> p k i", p=128))
                xab.append(xa)
            for half, xa in enumerate(xab):
                # merged [Q^T | K^T] for this batch: full 128-wide stationary
                ps = pps.tile([128, 512], dt.float32, tag="qk")
                for k in range(cch):
                    nc.tensor.matmul(ps[:], wqk_sb[:, k, :], xa[:, k, :],
                                     start=(k == 0), stop=(k == cch - 1))
                stg = stgp.tile([128, 512], dt.bfloat16, tag="stg")
                nc.vector.tensor_copy(stg[:], ps[:])
                # redistribute to batch-packed partition halves (cross-partition)
                nc.sync.dma_start(qt2[half * DH:(half + 1) * DH,
                                      c * 512:(c + 1) * 512], stg[0:DH, :])
                nc.sync.dma_start(kt2[half * DH:(half + 1) * DH,
                                      c * 512:(c + 1) * 512], stg[DH:128, :])
                # V: [m, d] tiles, one per 128 tokens; ones column appended
                psv = vps.tile([128, 4, DH + 2], dt.float32, tag="v")
                for mt in range(4):
                    for k in range(cch):
                        nc.tensor.matmul(psv[:, mt, 0:DH],
                                         xa[:, k, mt * 128:(mt + 1) * 128],
                                         wv_sb[:, k, :],
                                         start=(k == 0), stop=(k == cch - 1))
                nc.vector.memset(psv[:, :, DH:DH + 1], 1.0)
                nc.vector.tensor_copy(
                    vext[half][:].rearrange(
                        "p (t c) -> p t c", c=DH + 1)[:, c * 4:(c + 1) * 4, :],
                    psv[:, :, 0:DH + 1])

        def emit_proj(nb, jps):
            """Projection + bf16 store for query block nb."""
            for t in range(tpb):
                gt = nb * tpb + t
                pp = [jps.tile([128, C], dt.float32, tag=f"pp{j}", name=f"pp{j}")
                      for j in range(2)]
                nc.tensor.matmul(pp[0][:], otsb[0:DH, gt * 128:(gt + 1) * 128],
                                 wp_sb[0:DH, :], start=True, stop=True,
                                 tile_position=(0, 0))
                nc.tensor.matmul(pp[1][:], otsb[DH:128, gt * 128:(gt + 1) * 128],
                                 wp_sb[DH:128, :], start=True, stop=True,
                                 tile_position=(64, 0))
                for j in range(2):
                    ob = obp.tile([128, C], dt.bfloat16, tag=f"ob{j}", name="ob")
                    if j == 0:
                        nc.vector.tensor_copy(ob[:], pp[j][:])
                    else:
                        nc.scalar.copy(ob[:], pp[j][:])
                    nc.sync.dma_start(
                        out[j * n_seq + gt * 128: j * n_seq + (gt + 1) * 128, :],
                        ob[:])

        def emit_norm(nb, accp, dbs):
            """Deferred DVE half of block nb's epilogue: 1/D + normalize into
            otsb. Emitted a couple of key-chunks into block nb+1 so the DVE
            FIFO never stalls on the gpsimd broadcast round trip."""
            for j in range(2):
                rr = rrp.tile([DH, NB], dt.float32, tag="rr", name="rr")
                nc.vector.reciprocal_approx_fast(rr[:], dbs[j][:])
                nc.vector.scalar_tensor_tensor(
                    otsb[j * DH:(j + 1) * DH, nb * NB:(nb + 1) * NB],
                    accp[j][0:DH, :], TRUNC_COMP, rr[:], MUL, MUL)

        def emit_block(nb, prev, jps, prep_sched=None):
            """One query block's S^T/exp/AV stream with deferred epilogues.
            prep_sched: optional {mc: chunk} map of prep work to interleave."""
            acc = [aps.tile([DH + 1, NB], dt.float32, tag=f"acc{j}",
                            name=f"acc{j}") for j in range(2)]

            def emit_av(ppt, pmc):
                for j in range(2):
                    nc.tensor.matmul(
                        acc[j][:],
                        vext[j][:, pmc * (DH + 1):(pmc + 1) * (DH + 1)],
                        ppt[:, j * 512:j * 512 + NB],
                        start=(pmc == 0), stop=(pmc == nmc - 1))

            # key chunks processed in PAIRS: two row-tiled S^T pairs
            # back-to-back, then four full-array AV matmuls (four chunks
            # behind), so the PE switches tiling mode once per pair instead
            # of every chunk. Projection matmuls (row-tiled) are emitted
            # right after an S^T group, where the mode already matches.
            pending = []   # [(pt_tile, mc), ...]
            for mcp in range(0, nmc, 2):
                grp = []
                for mc in (mcp, mcp + 1):
                    st = sps.tile([128, 1024], dt.float32, tag="s")
                    for j in range(2):
                        nc.tensor.matmul(
                            st[:, j * 512:j * 512 + NB],
                            kt2[j * DH:(j + 1) * DH, mc * 128:(mc + 1) * 128],
                            qt2[j * DH:(j + 1) * DH, nb * NB:(nb + 1) * NB],
                            start=True, stop=True,
                            tile_position=(j * 64, 0))
                    grp.append((st, mc))
                if mcp == 4 and nb > 0:
                    emit_proj(nb - 1, jps)  # previous block's projection
                for st, mc in grp:
                    pt = ptp.tile([128, 1024], dt.bfloat16, tag="pt")
                    if act_mask[mc]:
                        nc.scalar.activation(pt[:], st[:], EXP, bias=0.0,
                                             scale=SCALE)
                    else:
                        nc.vector.tensor_scalar(pt[:].bitcast(dt.int16), st[:],
                                                SCH_A, SCH_B, MUL, ADD)
                    pending.append((pt, mc))
                while len(pending) > 4:
                    emit_av(*pending.pop(0))
                if prep_sched and mcp in prep_sched:
                    emit_prep(*prep_sched[mcp])
                if mcp == 2 and prev is not None:
                    emit_norm(*prev)    # previous block's normalize, deferred
                    prev = None
            for ppt, pmc in pending:
                emit_av(ppt, pmc)
            # epilogue (ACT + gpsimd legs now; DVE legs deferred into nb+1)
            dbs = []
            for j in range(2):
                dsb = rrp.tile([1, NB], dt.float32, tag="dsb", name="dsb")
                db = rrp.tile([DH, NB], dt.float32, tag="db", name="db")
                nc.scalar.copy(dsb[:], acc[j][DH:DH + 1, :])
                nc.gpsimd.partition_broadcast(db[:], dsb[:])
                dbs.append(db)
            return (nb, acc, dbs)

        with tc.tile_pool(name="xa", bufs=4) as xpool, \
             tc.tile_pool(name="stg", bufs=3) as stgp, \
             tc.tile_pool(name="prep_ps", bufs=1, space="PSUM") as pps, \
             tc.tile_pool(name="prep_v_ps", bufs=1, space="PSUM") as vps:
            pools = (xpool, stgp, pps, vps)
            for c in range(2):
                emit_prep(c, *pools)
            sched = {2 + 4 * i: (2 + i,) + pools for i in range(ntc - 2)}
            prev = emit_block(0, None, None, prep_sched=sched)
        with tc.tile_pool(name="proj_ps", bufs=1, space="PSUM") as jps:
            for nb in range(1, nblk):
                prev = emit_block(nb, prev, jps)
            emit_norm(*prev)
            emit_proj(nblk - 1, jps)

    persist.release()
    const.release()


def build_kernel(n_seq=N):
    nc = bacc.Bacc("TRN2", target_bir_lowering=False, debug=False, num_devices=8)
    dt = mybir.dt
    t_tot = 2 * n_seq
    xt = nc.dram_tensor("xt", [C, t_tot], dt.bfloat16, kind="ExternalInput").ap()
    wqk = nc.dram_tensor("wqk", [C, 128], dt.bfloat16, kind="ExternalInput").ap()
    wv = nc.dram_tensor("wv", [C, DH], dt.bfloat16, kind="ExternalInput").ap()
    wp2 = nc.dram_tensor("wp2", [128, C], dt.bfloat16, kind="ExternalInput").ap()
    out = nc.dram_tensor("out", [t_tot, C], dt.bfloat16, kind="ExternalOutput").ap()
    with tile.TileContext(nc) as tc:
        _attention_body(nc, tc, xt, wqk, wv, wp2, out, n_seq)
    nc.compile()
    return nc


def make_in_maps(x, w_qkv, w_proj, n_seq=N):
    """Slice the full inputs into 8 per-core input maps (head per core)."""
    t_tot = 2 * n_seq
    xt = np.ascontiguousarray(x.reshape(t_tot, C).T).astype(BF16)
    in_maps = []
    for h in range(H):
        wq = w_qkv[h * DH:(h + 1) * DH, :].T                      # [C, DH]
        wk = w_qkv[C + h * DH:C + (h + 1) * DH, :].T
        wqk = np.ascontiguousarray(
            np.concatenate([wq, wk], axis=1)).astype(BF16)        # [C, 128]
        wv = np.ascontiguousarray(
            w_qkv[2 * C + h * DH:2 * C + (h + 1) * DH, :].T).astype(BF16)
        wp = np.ascontiguousarray(w_proj[:, h * DH:(h + 1) * DH].T)  # [DH, C]
        wp2 = np.concatenate([wp, wp], axis=0).astype(BF16)          # [128, C]
        in_maps.append({"xt": xt, "wqk": wqk, "wv": wv, "wp2": wp2})
    return in_maps


_NC_CACHE = {}


def _get_nc(n_seq=N):
    if n_seq not in _NC_CACHE:
        _NC_CACHE[n_seq] = build_kernel(n_seq)
    return _NC_CACHE[n_seq]


def run(x, w_qkv, w_proj, b_proj, trace=False, tmpdir=None):
    x = np.asarray(x, dtype=np.float32)
    w_qkv = np.asarray(w_qkv, dtype=np.float32)
    w_proj = np.asarray(w_proj, dtype=np.float32)
    b_proj = np.asarray(b_proj, dtype=np.float32)
    nc = _get_nc()
    in_maps = make_in_maps(x, w_qkv, w_proj)
    try:
        res = run_bass_kernel_spmd(nc, in_maps, list(range(H)), trace=trace,
                                   tmpdir=tmpdir)
    except ModuleNotFoundError:
        res = run_bass_kernel_spmd(nc, in_maps, list(range(H)), trace=False,
                                   tmpdir=tmpdir)
    partial_sum = np.zeros((T, C), np.float32)
    for r in res.results:
        partial_sum += r["out"].astype(np.float32)
    full = partial_sum + b_proj[None, :]
    return full.reshape(B, N, C), res


def kernel(x, w_qkv, w_proj, b_proj):
    out, _ = run(x, w_qkv, w_proj, b_proj)
    return out


# revision 26
# speedup vs baseline: 1.5099x; 1.0109x over previous
"""Multi-head self-attention (B=2, N=4096, C=512, H=8) on 8 trn2 NeuronCores.

Sharding: one head per core (tensor parallel over heads). Each core:
  - computes Q^T,K^T (d-on-partitions, two batches packed on partition halves)
    via one merged [wq|wk] stationary (full 128-wide PE) + a DMA redistribute,
    and V (m-on-partitions) for its head from the full token stream,
  - runs flash-style attention per batch: S^T = K Q^T via row-tiled matmul
    pairs; the softmax exp is SPLIT between the scalar (ACT) engine (exact
    exp) and the vector engine (Schraudolph bitcast exp2: one tensor_scalar
    mult+add whose int16 output bits form the bf16 of 2^(A*s+B)), so both
    engines stream the N^2 softmax concurrently; AV matmuls run two key-chunks
    behind the exp so the PE never stalls on it,
  - P^T @ [V|1] accumulated in PSUM, the appended ones-column producing the
    softmax denominators for free,
  - normalizes the head output *before* projection (vector stt multiply with
    a gpsimd-broadcast reciprocal), projects through the head's w_proj slice
    with row-packed matmul pairs, and DMA-truncates the fp32 PSUM result
    straight to bf16 DRAM partials (truncation bias compensated in the
    normalize multiply).
Host sums the 8 bf16 partials in fp32 and adds b_proj.
"""

import numpy as np
import ml_dtypes

import concourse.bacc as bacc
import concourse.bass as bass
import concourse.mybir as mybir
import concourse.tile as tile
from concourse.bass_utils import run_bass_kernel_spmd

BF16 = ml_dtypes.bfloat16

B = 2
N = 4096          # sequence length per batch
C = 512           # channels
H = 8             # heads
DH = C // H       # 64 head dim
T = B * N         # total tokens
NB = 512          # query-block size
MC = 128          # key-chunk size
SCALE = float(DH) ** -0.5

# Schraudolph bitcast exp2 constants (bf16 target):
#   bf16 bits of exp(SCALE*s)  ~=  round(SCH_A*s + SCH_B)
SCH_A = 128.0 * SCALE * 1.4426950408889634
SCH_B = 16256.0 - 128.0 * 0.045

# engine casts round; no truncation compensation needed
TRUNC_COMP = 1.0

N_ACT = 18        # of every 32 key chunks, this many exp'd on ACT (rest DVE)


def _act_mask(n_act, n=32):
    return [((mc + 1) * n_act) // n - (mc * n_act) // n > 0 for mc in range(n)]


def _attention_body(nc, tc, xt, wqk, wv, wp2, out, n_seq):
    dt = mybir.dt
    cch = C // 128             # contraction chunks over C
    nblk = n_seq // NB         # query blocks per batch
    nmc = n_seq // MC          # key chunks per batch
    ntc = n_seq // 512         # 512-token chunks per batch (qkv prep)
    tpb = NB // 128            # 128-token proj chunks per query block
    EXP = mybir.ActivationFunctionType.Exp
    MUL = mybir.AluOpType.mult
    ADD = mybir.AluOpType.add
    act_mask = _act_mask(N_ACT if nmc == 32 else max(1, int(round(N_ACT * nmc / 32.0))),
                         nmc)

    const = tc.alloc_tile_pool(name="const", bufs=1)
    persist = tc.alloc_tile_pool(name="persist", bufs=1)

    # constants
    wqk_sb = const.tile([128, cch, 128], dt.bfloat16)
    wv_sb = const.tile([128, cch, DH], dt.bfloat16)
    nc.sync.dma_start(wqk_sb[:], wqk.rearrange("(c p) d -> p c d", p=128))
    nc.sync.dma_start(wv_sb[:], wv.rearrange("(c p) d -> p c d", p=128))
    wp_sb = const.tile([128, C], dt.bfloat16)     # wp duplicated on both halves
    nc.sync.dma_start(wp_sb[:], wp2)

    # persistent per-head tensors
    qt2 = persist.tile([128, n_seq], dt.bfloat16)   # rows 0:64 b0 Q^T, 64:128 b1
    kt2 = persist.tile([128, n_seq], dt.bfloat16)
    vext = [persist.tile([128, nmc * (DH + 1)], dt.bfloat16, name=f"vext{j}")
            for j in range(2)]
    otsb = persist.tile([128, n_seq], dt.bfloat16)  # rows 0:64 b0 head-out, 64:128 b1

    # fused structure: QKV prep is interleaved into query-block 0's attention
    # stream so the exp engines start working a few µs in; the prep PSUM pools
    # close before the first projection so their space is recycled for it.
    with tc.tile_pool(name="s_ps", bufs=2, space="PSUM") as sps, \
         tc.tile_pool(name="acc_ps", bufs=1, space="PSUM") as aps, \
         tc.tile_pool(name="ptp", bufs=4) as ptp, \
         tc.tile_pool(name="obp", bufs=2) as obp, \
         tc.tile_pool(name="rrp", bufs=2) as rrp:

        def emit_prep(c, xpool, stgp, pps, vps):
            """QKV prep for one 512-token chunk of both batches."""
            xab = []
            for half in range(2):
                xa = xpool.tile([128, cch, 512], dt.bfloat16, tag="x")
                nc.sync.dma_start(
                    xa[:], xt[:, half * n_seq + c * 512:half * n_seq + (c + 1) * 512]
                    .rearrange("(k p) i -> p k i", p=128))
                xab.append(xa)
            for half, xa in enumerate(xab):
                # merged [Q^T | K^T] for this batch: full 128-wide stationary
                ps = pps.tile([128, 512], dt.float32, tag="qk")
                for k in range(cch):
                    nc.tensor.matmul(ps[:], wqk_sb[:, k, :], xa[:, k, :],
                                     start=(k == 0), stop=(k == cch - 1))
                stg = stgp.tile([128, 512], dt.bfloat16, tag="stg")
                nc.vector.tensor_copy(stg[:], ps[:])
                # redistribute to batch-packed partition halves (cross-partition)
                nc.sync.dma_start(qt2[half * DH:(half + 1) * DH,
                                      c * 512:(c + 1) * 512], stg[0:DH, :])
                nc.sync.dma_start(kt2[half * DH:(half + 1) * DH,
                                      c * 512:(c + 1) * 512], stg[DH:128, :])
                # V: [m, d] tiles, one per 128 tokens; ones column appended
                psv = vps.tile([128, 4, DH + 2], dt.float32, tag="v")
                for mt in range(4):
                    for k in range(cch):
                        nc.tensor.matmul(psv[:, mt, 0:DH],
                                         xa[:, k, mt * 128:(mt + 1) * 128],
                                         wv_sb[:, k, :],
                                         start=(k == 0), stop=(k == cch - 1))
                nc.vector.memset(psv[:, :, DH:DH + 1], 1.0)
                nc.vector.tensor_copy(
                    vext[half][:].rearrange(
                        "p (t c) -> p t c", c=DH + 1)[:, c * 4:(c + 1) * 4, :],
                    psv[:, :, 0:DH + 1])

        def emit_proj(nb, jps):
            """Projection + bf16 store for query block nb."""
            for t in range(tpb):
                gt = nb * tpb + t
                pp = [jps.tile([128, C], dt.float32, tag=f"pp{j}", name=f"pp{j}")
                      for j in range(2)]
                nc.tensor.matmul(pp[0][:], otsb[0:DH, gt * 128:(gt + 1) * 128],
                                 wp_sb[0:DH, :], start=True, stop=True,
                                 tile_position=(0, 0))
                nc.tensor.matmul(pp[1][:], otsb[DH:128, gt * 128:(gt + 1) * 128],
                                 wp_sb[DH:128, :], start=True, stop=True,
                                 tile_position=(64, 0))
                for j in range(2):
                    ob = obp.tile([128, C], dt.bfloat16, tag=f"ob{j}", name="ob")
                    if j == 0:
                        nc.vector.tensor_copy(ob[:], pp[j][:])
                    else:
                        nc.scalar.copy(ob[:], pp[j][:])
                    nc.sync.dma_start(
                        out[j * n_seq + gt * 128: j * n_seq + (gt + 1) * 128, :],
                        ob[:])

        def emit_norm(nb, accp, dbs):
            """Deferred DVE half of block nb's epilogue: 1/D + normalize into
            otsb. Emitted a couple of key-chunks into block nb+1 so the DVE
            FIFO never stalls on the gpsimd broadcast round trip."""
            for j in range(2):
                rr = rrp.tile([DH, NB], dt.float32, tag="rr", name="rr")
                nc.vector.reciprocal_approx_fast(rr[:], dbs[j][:])
                nc.vector.scalar_tensor_tensor(
                    otsb[j * DH:(j + 1) * DH, nb * NB:(nb + 1) * NB],
                    accp[j][0:DH, :], TRUNC_COMP, rr[:], MUL, MUL)

        def emit_block(nb, prev, jps, prep_sched=None):
            """One query block's S^T/exp/AV stream with deferred epilogues.
            prep_sched: optional {mc: chunk} map of prep work to interleave."""
            acc = [aps.tile([DH + 1, NB], dt.float32, tag=f"acc{j}",
                            name=f"acc{j}") for j in range(2)]
            # AV runs TWO steps behind S^T/exp so the PE never waits on exp.
            pending = []   # [(pt_tile, mc), ...]
            for mc in range(nmc):
                st = sps.tile([128, 1024], dt.float32, tag="s")
                for j in range(2):
                    nc.tensor.matmul(
                        st[:, j * 512:j * 512 + NB],
                        kt2[j * DH:(j + 1) * DH, mc * 128:(mc + 1) * 128],
                        qt2[j * DH:(j + 1) * DH, nb * NB:(nb + 1) * NB],
                        start=True, stop=True,
                        tile_position=(j * 64, 0))
                if mc == 0 and prev is not None:
                    # previous block's deferred normalize, emitted before this
                    # block's first DVE exp: DVE drains it while ACT handles
                    # the first chunks, and the acc banks free before AV(0).
                    emit_norm(*prev)
                    prev = None
                pt = ptp.tile([128, 1024], dt.bfloat16, tag="pt")
                if act_mask[mc]:
                    nc.scalar.activation(pt[:], st[:], EXP, bias=0.0, scale=SCALE)
                else:
                    nc.vector.tensor_scalar(pt[:].bitcast(dt.int16), st[:],
                                            SCH_A, SCH_B, MUL, ADD)
                pending.append((pt, mc))
                if len(pending) > 2 or (mc == nmc - 1):
                    todo = pending if mc == nmc - 1 else [pending.pop(0)]
                    for ppt, pmc in todo:
                        for j in range(2):
                            nc.tensor.matmul(
                                acc[j][:],
                                vext[j][:, pmc * (DH + 1):(pmc + 1) * (DH + 1)],
                                ppt[:, j * 512:j * 512 + NB],
                                start=(pmc == 0), stop=(pmc == nmc - 1))
                if prep_sched and mc in prep_sched:
                    emit_prep(*prep_sched[mc])
                if mc == 5 and nb > 0:
                    emit_proj(nb - 1, jps)  # previous block's projection
            # epilogue (ACT + gpsimd legs now; DVE legs deferred into nb+1)
            dbs = []
            for j in range(2):
                dsb = rrp.tile([1, NB], dt.float32, tag="dsb", name="dsb")
                db = rrp.tile([DH, NB], dt.float32, tag="db", name="db")
                nc.scalar.copy(dsb[:], acc[j][DH:DH + 1, :])
                nc.gpsimd.partition_broadcast(db[:], dsb[:])
                dbs.append(db)
            return (nb, acc, dbs)

        with tc.tile_pool(name="xa", bufs=6) as xpool, \
             tc.tile_pool(name="stg", bufs=3) as stgp, \
             tc.tile_pool(name="prep_ps", bufs=1, space="PSUM") as pps, \
             tc.tile_pool(name="prep_v_ps", bufs=1, space="PSUM") as vps:
            pools = (xpool, stgp, pps, vps)
            for c in range(2):
                emit_prep(c, *pools)
            sched = {3 + 4 * i: (2 + i,) + pools for i in range(ntc - 2)}
            prev = emit_block(0, None, None, prep_sched=sched)
        with tc.tile_pool(name="proj_ps", bufs=1, space="PSUM") as jps:
            for nb in range(1, nblk):
                prev = emit_block(nb, prev, jps)
            emit_norm(*prev)
            emit_proj(nblk - 1, jps)

    persist.release()
    const.release()


def build_kernel(n_seq=N):
    nc = bacc.Bacc("TRN2", target_bir_lowering=False, debug=False, num_devices=8)
    dt = mybir.dt
    t_tot = 2 * n_seq
    xt = nc.dram_tensor("xt", [C, t_tot], dt.bfloat16, kind="ExternalInput").ap()
    wqk = nc.dram_tensor("wqk", [C, 128], dt.bfloat16, kind="ExternalInput").ap()
    wv = nc.dram_tensor("wv", [C, DH], dt.bfloat16, kind="ExternalInput").ap()
    wp2 = nc.dram_tensor("wp2", [128, C], dt.bfloat16, kind="ExternalInput").ap()
    out = nc.dram_tensor("out", [t_tot, C], dt.bfloat16, kind="ExternalOutput").ap()
    with tile.TileContext(nc) as tc:
        _attention_body(nc, tc, xt, wqk, wv, wp2, out, n_seq)
    nc.compile()
    return nc


def make_in_maps(x, w_qkv, w_proj, n_seq=N):
    """Slice the full inputs into 8 per-core input maps (head per core)."""
    t_tot = 2 * n_seq
    xt = np.ascontiguousarray(x.reshape(t_tot, C).T).astype(BF16)
    in_maps = []
    for h in range(H):
        wq = w_qkv[h * DH:(h + 1) * DH, :].T                      # [C, DH]
        wk = w_qkv[C + h * DH:C + (h + 1) * DH, :].T
        wqk = np.ascontiguousarray(
            np.concatenate([wq, wk], axis=1)).astype(BF16)        # [C, 128]
        wv = np.ascontiguousarray(
            w_qkv[2 * C + h * DH:2 * C + (h + 1) * DH, :].T).astype(BF16)
        wp = np.ascontiguousarray(w_proj[:, h * DH:(h + 1) * DH].T)  # [DH, C]
        wp2 = np.concatenate([wp, wp], axis=0).astype(BF16)          # [128, C]
        in_maps.append({"xt": xt, "wqk": wqk, "wv": wv, "wp2": wp2})
    return in_maps


_NC_CACHE = {}


def _get_nc(n_seq=N):
    if n_seq not in _NC_CACHE:
        _NC_CACHE[n_seq] = build_kernel(n_seq)
    return _NC_CACHE[n_seq]


def run(x, w_qkv, w_proj, b_proj, trace=False, tmpdir=None):
    x = np.asarray(x, dtype=np.float32)
    w_qkv = np.asarray(w_qkv, dtype=np.float32)
    w_proj = np.asarray(w_proj, dtype=np.float32)
    b_proj = np.asarray(b_proj, dtype=np.float32)
    nc = _get_nc()
    in_maps = make_in_maps(x, w_qkv, w_proj)
    try:
        res = run_bass_kernel_spmd(nc, in_maps, list(range(H)), trace=trace,
                                   tmpdir=tmpdir)
    except ModuleNotFoundError:
        res = run_bass_kernel_spmd(nc, in_maps, list(range(H)), trace=False,
                                   tmpdir=tmpdir)
    partial_sum = np.zeros((T, C), np.float32)
    for r in res.results:
        partial_sum += r["out"].astype(np.float32)
    full = partial_sum + b_proj[None, :]
    return full.reshape(B, N, C), res


def kernel(x, w_qkv, w_proj, b_proj):
    out, _ = run(x, w_qkv, w_proj, b_proj)
    return out


# revision 27
# speedup vs baseline: 1.5270x; 1.0113x over previous
"""Multi-head self-attention (B=2, N=4096, C=512, H=8) on 8 trn2 NeuronCores.

Sharding: one head per core (tensor parallel over heads). Each core:
  - computes Q^T,K^T (d-on-partitions, two batches packed on partition halves)
    via one merged [wq|wk] stationary (full 128-wide PE) + a DMA redistribute,
    and V (m-on-partitions) for its head from the full token stream,
  - runs flash-style attention per batch: S^T = K Q^T via row-tiled matmul
    pairs; the softmax exp is SPLIT between the scalar (ACT) engine (exact
    exp) and the vector engine (Schraudolph bitcast exp2: one tensor_scalar
    mult+add whose int16 output bits form the bf16 of 2^(A*s+B)), so both
    engines stream the N^2 softmax concurrently; AV matmuls run two key-chunks
    behind the exp so the PE never stalls on it,
  - P^T @ [V|1] accumulated in PSUM, the appended ones-column producing the
    softmax denominators for free,
  - normalizes the head output *before* projection (vector stt multiply with
    a gpsimd-broadcast reciprocal), projects through the head's w_proj slice
    with row-packed matmul pairs, and DMA-truncates the fp32 PSUM result
    straight to bf16 DRAM partials (truncation bias compensated in the
    normalize multiply).
Host sums the 8 bf16 partials in fp32 and adds b_proj.
"""

import numpy as np
import ml_dtypes

import concourse.bacc as bacc
import concourse.bass as bass
import concourse.mybir as mybir
import concourse.tile as tile
from concourse.bass_utils import run_bass_kernel_spmd

BF16 = ml_dtypes.bfloat16

B = 2
N = 4096          # sequence length per batch
C = 512           # channels
H = 8             # heads
DH = C // H       # 64 head dim
T = B * N         # total tokens
NB = 512          # query-block size
MC = 128          # key-chunk size
SCALE = float(DH) ** -0.5

# Schraudolph bitcast exp2 constants (bf16 target):
#   bf16 bits of exp(SCALE*s)  ~=  round(SCH_A*s + SCH_B)
SCH_A = 128.0 * SCALE * 1.4426950408889634
SCH_B = 16256.0 - 128.0 * 0.045

# engine casts round; no truncation compensation needed
TRUNC_COMP = 1.0

N_ACT = 18        # of every 32 key chunks, this many exp'd on ACT (rest DVE)


def _act_mask(n_act, n=32):
    return [((mc + 1) * n_act) // n - (mc * n_act) // n > 0 for mc in range(n)]


def _attention_body(nc, tc, xt, wqk, wv, wp2, out, n_seq):
    dt = mybir.dt
    cch = C // 128             # contraction chunks over C
    nblk = n_seq // NB         # query blocks per batch
    nmc = n_seq // MC          # key chunks per batch
    ntc = n_seq // 512         # 512-token chunks per batch (qkv prep)
    tpb = NB // 128            # 128-token proj chunks per query block
    EXP = mybir.ActivationFunctionType.Exp
    MUL = mybir.AluOpType.mult
    ADD = mybir.AluOpType.add
    act_mask = _act_mask(N_ACT if nmc == 32 else max(1, int(round(N_ACT * nmc / 32.0))),
                         nmc)

    const = tc.alloc_tile_pool(name="const", bufs=1)
    persist = tc.alloc_tile_pool(name="persist", bufs=1)

    # constants
    wqk_sb = const.tile([128, cch, 128], dt.bfloat16)
    wv_sb = const.tile([128, cch, DH], dt.bfloat16)
    nc.sync.dma_start(wqk_sb[:], wqk.rearrange("(c p) d -> p c d", p=128))
    nc.sync.dma_start(wv_sb[:], wv.rearrange("(c p) d -> p c d", p=128))
    wp_sb = const.tile([128, C], dt.bfloat16)     # wp duplicated on both halves
    nc.sync.dma_start(wp_sb[:], wp2)

    # persistent per-head tensors
    qt2 = persist.tile([128, n_seq], dt.bfloat16)   # rows 0:64 b0 Q^T, 64:128 b1
    kt2 = persist.tile([128, n_seq], dt.bfloat16)
    vext = [persist.tile([128, nmc * (DH + 1)], dt.bfloat16, name=f"vext{j}")
            for j in range(2)]
    otsb = persist.tile([128, n_seq], dt.bfloat16)  # rows 0:64 b0 head-out, 64:128 b1

    # fused structure: QKV prep is interleaved into query-block 0's attention
    # stream so the exp engines start working a few µs in; the prep PSUM pools
    # close before the first projection so their space is recycled for it.
    with tc.tile_pool(name="s_ps", bufs=2, space="PSUM") as sps, \
         tc.tile_pool(name="acc_ps", bufs=1, space="PSUM") as aps, \
         tc.tile_pool(name="ptp", bufs=6) as ptp, \
         tc.tile_pool(name="obp", bufs=2) as obp, \
         tc.tile_pool(name="rrp", bufs=2) as rrp:

        def emit_prep(c, xpool, stgp, pps, vps):
            """QKV prep for one 512-token chunk of both batches."""
            xab = []
            for half in range(2):
                xa = xpool.tile([128, cch, 512], dt.bfloat16, tag="x")
                nc.sync.dma_start(
                    xa[:], xt[:, half * n_seq + c * 512:half * n_seq + (c + 1) * 512]
                    .rearrange("(k p) i -> p k i", p=128))
                xab.append(xa)
            for half, xa in enumerate(xab):
                # merged [Q^T | K^T] for this batch: full 128-wide stationary
                ps = pps.tile([128, 512], dt.float32, tag="qk")
                for k in range(cch):
                    nc.tensor.matmul(ps[:], wqk_sb[:, k, :], xa[:, k, :],
                                     start=(k == 0), stop=(k == cch - 1))
                stg = stgp.tile([128, 512], dt.bfloat16, tag="stg")
                nc.vector.tensor_copy(stg[:], ps[:])
                # redistribute to batch-packed partition halves (cross-partition)
                nc.sync.dma_start(qt2[half * DH:(half + 1) * DH,
                                      c * 512:(c + 1) * 512], stg[0:DH, :])
                nc.sync.dma_start(kt2[half * DH:(half + 1) * DH,
                                      c * 512:(c + 1) * 512], stg[DH:128, :])
                # V: [m, d] tiles, one per 128 tokens; ones column appended
                psv = vps.tile([128, 4, DH + 2], dt.float32, tag="v")
                for mt in range(4):
                    for k in range(cch):
                        nc.tensor.matmul(psv[:, mt, 0:DH],
                                         xa[:, k, mt * 128:(mt + 1) * 128],
                                         wv_sb[:, k, :],
                                         start=(k == 0), stop=(k == cch - 1))
                nc.vector.memset(psv[:, :, DH:DH + 1], 1.0)
                nc.vector.tensor_copy(
                    vext[half][:].rearrange(
                        "p (t c) -> p t c", c=DH + 1)[:, c * 4:(c + 1) * 4, :],
                    psv[:, :, 0:DH + 1])

        def emit_proj(nb, jps):
            """Projection + bf16 store for query block nb."""
            for t in range(tpb):
                gt = nb * tpb + t
                pp = [jps.tile([128, C], dt.float32, tag=f"pp{j}", name=f"pp{j}")
                      for j in range(2)]
                nc.tensor.matmul(pp[0][:], otsb[0:DH, gt * 128:(gt + 1) * 128],
                                 wp_sb[0:DH, :], start=True, stop=True,
                                 tile_position=(0, 0))
                nc.tensor.matmul(pp[1][:], otsb[DH:128, gt * 128:(gt + 1) * 128],
                                 wp_sb[DH:128, :], start=True, stop=True,
                                 tile_position=(64, 0))
                for j in range(2):
                    ob = obp.tile([128, C], dt.bfloat16, tag=f"ob{j}", name="ob")
                    if j == 0:
                        nc.vector.tensor_copy(ob[:], pp[j][:])
                    else:
                        nc.scalar.copy(ob[:], pp[j][:])
                    nc.sync.dma_start(
                        out[j * n_seq + gt * 128: j * n_seq + (gt + 1) * 128, :],
                        ob[:])

        def emit_norm(nb, accp, dbs):
            """Deferred DVE half of block nb's epilogue: 1/D + normalize into
            otsb. Emitted a couple of key-chunks into block nb+1 so the DVE
            FIFO never stalls on the gpsimd broadcast round trip."""
            for j in range(2):
                rr = rrp.tile([DH, NB], dt.float32, tag="rr", name="rr")
                nc.vector.reciprocal_approx_fast(rr[:], dbs[j][:])
                nc.vector.scalar_tensor_tensor(
                    otsb[j * DH:(j + 1) * DH, nb * NB:(nb + 1) * NB],
                    accp[j][0:DH, :], TRUNC_COMP, rr[:], MUL, MUL)

        def emit_block(nb, prev, jps, prep_sched=None):
            """One query block's S^T/exp/AV stream with deferred epilogues.
            prep_sched: optional {mc: chunk} map of prep work to interleave."""
            acc = [aps.tile([DH + 1, NB], dt.float32, tag=f"acc{j}",
                            name=f"acc{j}") for j in range(2)]
            # AV runs TWO steps behind S^T/exp so the PE never waits on exp.
            pending = []   # [(pt_tile, mc), ...]
            for mc in range(nmc):
                st = sps.tile([128, 1024], dt.float32, tag="s")
                for j in range(2):
                    nc.tensor.matmul(
                        st[:, j * 512:j * 512 + NB],
                        kt2[j * DH:(j + 1) * DH, mc * 128:(mc + 1) * 128],
                        qt2[j * DH:(j + 1) * DH, nb * NB:(nb + 1) * NB],
                        start=True, stop=True,
                        tile_position=(j * 64, 0))
                if mc == 0 and prev is not None:
                    # previous block's deferred normalize, emitted before this
                    # block's first DVE exp: DVE drains it while ACT handles
                    # the first chunks, and the acc banks free before AV(0).
                    emit_norm(*prev)
                    prev = None
                pt = ptp.tile([128, 1024], dt.bfloat16, tag="pt")
                if act_mask[mc]:
                    nc.scalar.activation(pt[:], st[:], EXP, bias=0.0, scale=SCALE)
                else:
                    nc.vector.tensor_scalar(pt[:].bitcast(dt.int16), st[:],
                                            SCH_A, SCH_B, MUL, ADD)
                pending.append((pt, mc))
                if len(pending) > 3 or (mc == nmc - 1):
                    todo = pending if mc == nmc - 1 else [pending.pop(0)]
                    for ppt, pmc in todo:
                        for j in range(2):
                            nc.tensor.matmul(
                                acc[j][:],
                                vext[j][:, pmc * (DH + 1):(pmc + 1) * (DH + 1)],
                                ppt[:, j * 512:j * 512 + NB],
                                start=(pmc == 0), stop=(pmc == nmc - 1))
                if prep_sched and mc in prep_sched:
                    emit_prep(*prep_sched[mc])
                if mc == 5 and nb > 0:
                    emit_proj(nb - 1, jps)  # previous block's projection
            # epilogue (ACT + gpsimd legs now; DVE legs deferred into nb+1)
            dbs = []
            for j in range(2):
                dsb = rrp.tile([1, NB], dt.float32, tag="dsb", name="dsb")
                db = rrp.tile([DH, NB], dt.float32, tag="db", name="db")
                nc.scalar.copy(dsb[:], acc[j][DH:DH + 1, :])
                nc.gpsimd.partition_broadcast(db[:], dsb[:])
                dbs.append(db)
            return (nb, acc, dbs)

        with tc.tile_pool(name="xa", bufs=6) as xpool, \
             tc.tile_pool(name="stg", bufs=3) as stgp, \
             tc.tile_pool(name="prep_ps", bufs=1, space="PSUM") as pps, \
             tc.tile_pool(name="prep_v_ps", bufs=1, space="PSUM") as vps:
            pools = (xpool, stgp, pps, vps)
            for c in range(2):
                emit_prep(c, *pools)
            sched = {3 + 4 * i: (2 + i,) + pools for i in range(ntc - 2)}
            prev = emit_block(0, None, None, prep_sched=sched)
        with tc.tile_pool(name="proj_ps", bufs=1, space="PSUM") as jps:
            for nb in range(1, nblk):
                prev = emit_block(nb, prev, jps)
            emit_norm(*prev)
            emit_proj(nblk - 1, jps)

    persist.release()
    const.release()


def build_kernel(n_seq=N):
    nc = bacc.Bacc("TRN2", target_bir_lowering=False, debug=False, num_devices=8)
    dt = mybir.dt
    t_tot = 2 * n_seq
    xt = nc.dram_tensor("xt", [C, t_tot], dt.bfloat16, kind="ExternalInput").ap()
    wqk = nc.dram_tensor("wqk", [C, 128], dt.bfloat16, kind="ExternalInput").ap()
    wv = nc.dram_tensor("wv", [C, DH], dt.bfloat16, kind="ExternalInput").ap()
    wp2 = nc.dram_tensor("wp2", [128, C], dt.bfloat16, kind="ExternalInput").ap()
    out = nc.dram_tensor("out", [t_tot, C], dt.bfloat16, kind="ExternalOutput").ap()
    with tile.TileContext(nc) as tc:
        _attention_body(nc, tc, xt, wqk, wv, wp2, out, n_seq)
    nc.compile()
    return nc


def make_in_maps(x, w_qkv, w_proj, n_seq=N):
    """Slice the full inputs into 8 per-core input maps (head per core)."""
    t_tot = 2 * n_seq
    xt = np.ascontiguousarray(x.reshape(t_tot, C).T).astype(BF16)
    in_maps = []
    for h in range(H):
        wq = w_qkv[h * DH:(h + 1) * DH, :].T                      # [C, DH]
        wk = w_qkv[C + h * DH:C + (h + 1) * DH, :].T
        wqk = np.ascontiguousarray(
            np.concatenate([wq, wk], axis=1)).astype(BF16)        # [C, 128]
        wv = np.ascontiguousarray(
            w_qkv[2 * C + h * DH:2 * C + (h + 1) * DH, :].T).astype(BF16)
        wp = np.ascontiguousarray(w_proj[:, h * DH:(h + 1) * DH].T)  # [DH, C]
        wp2 = np.concatenate([wp, wp], axis=0).astype(BF16)          # [128, C]
        in_maps.append({"xt": xt, "wqk": wqk, "wv": wv, "wp2": wp2})
    return in_maps


_NC_CACHE = {}


def _get_nc(n_seq=N):
    if n_seq not in _NC_CACHE:
        _NC_CACHE[n_seq] = build_kernel(n_seq)
    return _NC_CACHE[n_seq]


def run(x, w_qkv, w_proj, b_proj, trace=False, tmpdir=None):
    x = np.asarray(x, dtype=np.float32)
    w_qkv = np.asarray(w_qkv, dtype=np.float32)
    w_proj = np.asarray(w_proj, dtype=np.float32)
    b_proj = np.asarray(b_proj, dtype=np.float32)
    nc = _get_nc()
    in_maps = make_in_maps(x, w_qkv, w_proj)
    try:
        res = run_bass_kernel_spmd(nc, in_maps, list(range(H)), trace=trace,
                                   tmpdir=tmpdir)
    except ModuleNotFoundError:
        res = run_bass_kernel_spmd(nc, in_maps, list(range(H)), trace=False,
                                   tmpdir=tmpdir)
    partial_sum = np.zeros((T, C), np.float32)
    for r in res.results:
        partial_sum += r["out"].astype(np.float32)
    full = partial_sum + b_proj[None, :]
    return full.reshape(B, N, C), res


def kernel(x, w_qkv, w_proj, b_proj):
    out, _ = run(x, w_qkv, w_proj, b_proj)
    return out
